# revision 8
# baseline (speedup 1.0000x reference)
"""MLA forward Bass kernel for 8 TRN2 NeuronCores.

Sharding: pure query-row sharding. Core c handles batch b = c//4 and query rows
[sl*512, (sl+1)*512) with sl = c%4, for ALL 16 heads. Keys/values span the full
sequence, so the compressed-KV path (kvc, k_rope) is computed per-core for the
whole batch (replicated across the 4 cores that share a batch), while the Q
path, attention, and the output projection only cover the core's 512 query
rows. The full output-projection contraction (all 16 heads) is local, so no
cross-core reduction is needed: the host just concatenates the 8 row-blocks.

Layouts: everything TensorE-facing is kept transposed ([feature, seq]) so the
feature dim sits on partitions and matmuls contract over it. Softmax runs on
S^T tiles [k, q]: exp on ACT (no max-shift; scores are O(1) here), denominator
via DVE tile-adds + a ones-matmul partition reduction, normalization folded
into the PSUM->SBUF drain of the attention output. RoPE's rotate-half is a
constant 128x128 permutation matmul. All matmuls run as float32r.
"""

import os
import sys

for _p in ("/root/.axon_site/_ro/trn_rl_repo", "/opt/trn_rl_repo"):
    if os.path.isdir(_p) and _p not in sys.path:
        sys.path.insert(0, _p)

import numpy as np

import concourse.bass as bass
import concourse.tile as tile
from concourse import mybir
from concourse.bass_utils import run_bass_kernel_spmd

F32 = mybir.dt.float32
F32R = mybir.dt.float32r
BF16 = mybir.dt.bfloat16

D = 2048        # d_model
S = 2048        # seq len
B = 2           # batch
H = 16          # heads
HD = 128        # nope head dim
KV = 512        # kv lora rank
QL = 768        # q lora rank
RD = 64         # rope dim
EPS = 1e-6
SQ = 512        # query rows per core
N_CORES = 8
GROUPS = 4      # head groups of 4
GH = 4          # heads per group
SCALE = 1.0 / float(np.sqrt(HD + RD))

NKV = KV // 128   # 4 kv-lora chunks
NQL = QL // 128   # 6 q-lora chunks
NS = S // 512     # 4 seq blocks
NST = S // 128    # 16 seq tiles


# ---------------------------------------------------------------------------
# The walrus build in this container only encodes a single sync-wait on a
# Drain (TPB_CTRL) instruction, but TileContext._drain_and_barrier parks the
# whole global-clock wait set on the tail drain ("Too many sync wait
# commands"). Hoist the waits onto single-wait NOPs ahead of a bare drain.
def _patch_tile_drain():
    from bass_rust import ScopedClock

    def _drain_and_barrier(self, tick_clock, wait_clock):
        probe = self.nc.sync.nop(nofuse=True)
        wait_clock.add_sem_waits(
            probe.ins, ScopedClock({None: tick_clock.global_clock})
        )
        si = probe.ins.sync_info
        waits = list(si.on_wait) if si is not None else []
        if len(waits) > 1:
            probe.ins.sync_info = mybir.SyncInfo(on_wait=waits[:1], on_update=[])
            for w in waits[1:]:
                extra = self.nc.sync.nop(nofuse=True)
                extra.ins.sync_info = mybir.SyncInfo(on_wait=[w], on_update=[])
        self.nc.sync.drain()

        self.nc.all_engine_barrier()
        assert self.sems is not None
        popped = self.nc._tile_sem_poison_stack.pop()
        assert popped is self._sem_poison
        self.nc.clear_and_free_semaphores(list(self.sems.allocated().values()))
        self.nc.all_engine_barrier()

    tile.TileContext._drain_and_barrier = _drain_and_barrier


_patch_tile_drain()


def _r(ap):
    return ap.bitcast(F32R)


def build_nc():
    nc = bass.Bass()

    xT = nc.dram_tensor("xT", [D, S], F32, kind="ExternalInput")
    xqT = nc.dram_tensor("xqT", [D, SQ], F32, kind="ExternalInput")
    wcq = nc.dram_tensor("wcq", [D, QL], F32, kind="ExternalInput")
    wckv = nc.dram_tensor("wckv", [D, KV], F32, kind="ExternalInput")
    wkr2 = nc.dram_tensor("wkr2", [D, 128], F32, kind="ExternalInput")
    wdq = nc.dram_tensor("wdq", [QL, H * HD], F32, kind="ExternalInput")
    wdqr = nc.dram_tensor("wdqr", [QL, H * RD], F32, kind="ExternalInput")
    wdk = nc.dram_tensor("wdk", [KV, H * HD], F32, kind="ExternalInput")
    wdv = nc.dram_tensor("wdv", [KV, H * HD], F32, kind="ExternalInput")
    wo = nc.dram_tensor("wo", [H * HD, D], F32, kind="ExternalInput")
    gq = nc.dram_tensor("gq", [1, QL], F32, kind="ExternalInput")
    gkv = nc.dram_tensor("gkv", [1, KV], F32, kind="ExternalInput")
    cosk = nc.dram_tensor("cosk", [128, S], F32, kind="ExternalInput")
    sink = nc.dram_tensor("sink", [128, S], F32, kind="ExternalInput")
    cosq = nc.dram_tensor("cosq", [128, SQ], F32, kind="ExternalInput")
    sinq = nc.dram_tensor("sinq", [128, SQ], F32, kind="ExternalInput")
    rotp = nc.dram_tensor("rotp", [128, 128], F32, kind="ExternalInput")
    out = nc.dram_tensor("out", [SQ, D], F32, kind="ExternalOutput")
    debug = bool(int(os.environ.get("MLA_DEBUG", "0")))
    if debug:
        dbg_kvcT = nc.dram_tensor("dbg_kvcT", [KV, S], F32, kind="ExternalOutput")
        dbg_qcT = nc.dram_tensor("dbg_qcT", [QL, SQ], F32, kind="ExternalOutput")
        dbg_krT = nc.dram_tensor("dbg_krT", [128, S], F32, kind="ExternalOutput")
        dbg_oT = nc.dram_tensor("dbg_oT", [H * 128, SQ], F32, kind="ExternalOutput")

    with tile.TileContext(nc) as tc:
        _build_body(nc, tc, locals(), debug)
    _split_excess_waits(nc)
    return nc


# This walrus build encodes at most one sync-wait per engine instruction;
# hoist surplus waits onto single-wait NOPs right before the instruction on
# the same engine queue (in-order execution keeps the semantics identical).
def _split_excess_waits(nc, max_waits=1):
    n_nops = 0
    for f in nc.m.functions:
        for bb in f.blocks:
            out = []
            for ins in bb.instructions:
                si = ins.sync_info
                if si is not None:
                    sem = [w for w in si.on_wait if w.sync_type == "semaphore"]
                    other = [w for w in si.on_wait if w.sync_type != "semaphore"]
                    budget = max(max_waits - len(other), 0)
                    if len(sem) > budget:
                        extra, keep = sem[:-budget] if budget else sem, (
                            sem[-budget:] if budget else [])
                        for j, w in enumerate(extra):
                            nop = mybir.InstNoOp(
                                name=f"{ins.name}-wsplit{j}",
                                engine=ins.engine,
                                bass_nofuse=True,
                                sync_info=mybir.SyncInfo(
                                    on_wait=[w], on_update=[]),
                            )
                            out.append(nop)
                            n_nops += 1
                        ins.sync_info = mybir.SyncInfo(
                            on_wait=other + keep,
                            on_update=list(si.on_update))
                out.append(ins)
            bb.instructions = out
    return n_nops


def _build_body(nc, tc, t, debug=False):
    from contextlib import ExitStack

    ctx = ExitStack()
    with ctx:
        consts = ctx.enter_context(tc.tile_pool(name="consts", bufs=1))
        persist = ctx.enter_context(tc.tile_pool(name="persist", bufs=1))
        misc = ctx.enter_context(tc.tile_pool(name="misc", bufs=2))
        # PSUM pools: aux first (lives through whole kernel), then phase pools.
        aux_ps = ctx.enter_context(tc.tile_pool(name="aux_ps", bufs=2, space="PSUM"))

        # ---- constants -----------------------------------------------------
        ones128f = consts.tile([128, 1], F32)
        nc.vector.memset(ones128f, 1.0)
        ones128 = consts.tile([128, 1], F32R)
        nc.scalar.copy(ones128, ones128f)
        ones1f = consts.tile([1, 128], F32)
        nc.vector.memset(ones1f, 1.0)
        ones1 = consts.tile([1, 128], F32R)
        nc.scalar.copy(ones1, ones1f)
        gq_s = consts.tile([1, QL], F32R)
        nc.sync.dma_start(out=gq_s, in_=t["gq"][:, :].bitcast(F32R))
        gkv_s = consts.tile([1, KV], F32R)
        nc.sync.dma_start(out=gkv_s, in_=t["gkv"][:, :].bitcast(F32R))
        eps_s = consts.tile([1, 1], F32)
        nc.vector.memset(eps_s, EPS)
        rotp_s = consts.tile([128, 128], F32R)
        nc.sync.dma_start(out=rotp_s, in_=t["rotp"][:, :].bitcast(F32R))
        cosq_s = consts.tile([128, SQ], F32)
        nc.sync.dma_start(out=cosq_s, in_=t["cosq"][:, :])
        sinq_s = consts.tile([128, SQ], F32)
        nc.sync.dma_start(out=sinq_s, in_=t["sinq"][:, :])

        # ---- persistent tiles ---------------------------------------------
        kvcT = [persist.tile([128, S], F32R, tag=f"kvcT{c}", name=f"kvcT{c}") for c in range(NKV)]
        krT = persist.tile([128, S], F32R, tag="krT")
        qcT = [persist.tile([128, SQ], F32R, tag=f"qcT{c}", name=f"qcT{c}") for c in range(NQL)]
        oT = [persist.tile([128, SQ], F32R, tag=f"oT{h}", name=f"oT{h}") for h in range(H)]

        # ===================================================================
        # Phase 0: compress. kvcT/krT over full seq, qcT over own query rows.
        # ===================================================================
        with nc.named_scope("p0_compress", notify=True), \
             tc.tile_pool(name="misc0", bufs=2) as misc0, \
             tc.tile_pool(name="xtp", bufs=3) as xtp, \
             tc.tile_pool(name="wkvhold", bufs=1) as wkvhold, \
             tc.tile_pool(name="wstream", bufs=3) as wstream, \
             tc.tile_pool(name="acc_ps", bufs=6, space="PSUM") as acc_ps:
            # wckv/wkr are reused by all 4 seq blocks: load once, keep in SBUF
            wkv_h = [wkvhold.tile([128, KV], F32R, tag=f"wckv{d}", name=f"wckv{d}")
                     for d in range(16)]
            wkr_h = [wkvhold.tile([128, 128], F32R, tag=f"wkr{d}", name=f"wkr{d}")
                     for d in range(16)]
            for d in range(16):
                drow = slice(d * 128, (d + 1) * 128)
                nc.sync.dma_start(out=wkv_h[d], in_=t["wckv"][drow, :].bitcast(F32R))
                nc.sync.dma_start(out=wkr_h[d], in_=t["wkr2"][drow, :].bitcast(F32R))
            for sb in range(NS):
                scol = slice(sb * 512, (sb + 1) * 512)
                pkv = [acc_ps.tile([128, 512], F32, tag="acc", name="pkv") for _ in range(NKV)]
                pkr = acc_ps.tile([128, 512], F32, tag="acc")
                for d in range(16):
                    drow = slice(d * 128, (d + 1) * 128)
                    xt = xtp.tile([128, 512], F32R, tag="xt")
                    nc.sync.dma_start(out=xt, in_=t["xT"][drow, scol].bitcast(F32R))
                    for c in range(NKV):
                        nc.tensor.matmul(
                            pkv[c], _r(wkv_h[d][:, c * 128:(c + 1) * 128]), _r(xt),
                            start=(d == 0), stop=(d == 15))
                    nc.tensor.matmul(pkr, _r(wkr_h[d]), _r(xt),
                                     start=(d == 0), stop=(d == 15))

                # rmsnorm over kv features (partition dim across the 4 chunks)
                ssq = aux_ps.tile([1, 512], F32, tag="aux")
                for c in range(NKV):
                    sq = misc0.tile([128, 512], F32R, tag="sq")
                    nc.scalar.square(sq, pkv[c])
                    nc.tensor.matmul(ssq, _r(ones128), _r(sq),
                                     start=(c == 0), stop=(c == NKV - 1))
                rstd = misc0.tile([1, 512], F32R, tag="rstd")
                nc.scalar.activation(rstd, ssq,
                                     mybir.ActivationFunctionType.Sqrt,
                                     bias=eps_s[:, :], scale=1.0 / KV)
                with nc.allow_low_precision(reason="f32r is full fp32 bits"):
                    nc.vector.reciprocal(rstd, rstd)
                for c in range(NKV):
                    bc = aux_ps.tile([128, 512], F32, tag="aux")
                    nc.tensor.matmul(
                        bc, _r(gkv_s[:, c * 128:(c + 1) * 128]), _r(rstd))
                    bc_s = misc.tile([128, 512], F32, tag="bcs")
                    nc.scalar.copy(bc_s, bc)
                    nc.vector.tensor_mul(kvcT[c][:, scol], pkv[c], bc_s)

                # rope on the (duplicated-rows) k_rope block
                ck = misc0.tile([128, 512], F32, tag="ck")
                nc.sync.dma_start(out=ck, in_=t["cosk"][:, scol])
                sk = misc0.tile([128, 512], F32, tag="sk")
                nc.sync.dma_start(out=sk, in_=t["sink"][:, scol])
                kraw = misc0.tile([128, 512], F32R, tag="kraw")
                nc.scalar.copy(kraw, pkr)
                rot = aux_ps.tile([128, 512], F32, tag="aux")
                nc.tensor.matmul(rot, _r(rotp_s), _r(kraw))
                t1 = misc0.tile([128, 512], F32, tag="ropet1")
                nc.vector.tensor_mul(t1, kraw, ck)
                t2 = misc0.tile([128, 512], F32, tag="ropet2")
                nc.vector.tensor_mul(t2, rot, sk)
                nc.vector.tensor_add(krT[:, scol], t1, t2)

            # qcT over own query rows
            pqc = [acc_ps.tile([128, 512], F32, tag="acc", name="pqc") for _ in range(NQL)]
            for d in range(16):
                drow = slice(d * 128, (d + 1) * 128)
                xt = xtp.tile([128, 512], F32R, tag="xt")
                nc.sync.dma_start(out=xt, in_=t["xqT"][drow, :].bitcast(F32R))
                wq_t = wstream.tile([128, QL], F32R, tag="wcq")
                nc.sync.dma_start(out=wq_t, in_=t["wcq"][drow, :].bitcast(F32R))
                for c in range(NQL):
                    nc.tensor.matmul(
                        pqc[c], _r(wq_t[:, c * 128:(c + 1) * 128]), _r(xt),
                        start=(d == 0), stop=(d == 15))
            ssq = aux_ps.tile([1, 512], F32, tag="aux")
            for c in range(NQL):
                sq = misc0.tile([128, 512], F32R, tag="sq")
                nc.scalar.square(sq, pqc[c])
                nc.tensor.matmul(ssq, _r(ones128), _r(sq),
                                 start=(c == 0), stop=(c == NQL - 1))
            rstd = misc0.tile([1, 512], F32R, tag="rstd")
            nc.scalar.activation(rstd, ssq, mybir.ActivationFunctionType.Sqrt,
                                 bias=eps_s[:, :], scale=1.0 / QL)
            with nc.allow_low_precision(reason="f32r is full fp32 bits"):
                nc.vector.reciprocal(rstd, rstd)
            for c in range(NQL):
                bc = aux_ps.tile([128, 512], F32, tag="aux")
                nc.tensor.matmul(bc, _r(gq_s[:, c * 128:(c + 1) * 128]), _r(rstd))
                bc_s = misc.tile([128, 512], F32, tag="bcs")
                nc.scalar.copy(bc_s, bc)
                nc.vector.tensor_mul(qcT[c], pqc[c], bc_s)

        # ===================================================================
        # Phase A: per head group -- decompress k/v/q, attention.
        # ===================================================================
        with nc.named_scope("pA_attn", notify=True), \
             tc.tile_pool(name="vpool", bufs=24) as vpool, \
             tc.tile_pool(name="khp", bufs=2) as khp, \
             tc.tile_pool(name="qnp", bufs=2) as qnp, \
             tc.tile_pool(name="qrp", bufs=2) as qrp, \
             tc.tile_pool(name="ptp", bufs=4) as ptp, \
             tc.tile_pool(name="denp", bufs=2) as denp, \
             tc.tile_pool(name="wdqp", bufs=6) as wdqp, \
             tc.tile_pool(name="wdqrp", bufs=6) as wdqrp, \
             tc.tile_pool(name="wdkp", bufs=4) as wdkp, \
             tc.tile_pool(name="wdvp", bufs=4) as wdvp, \
             tc.tile_pool(name="st_ps", bufs=2, space="PSUM") as st_ps, \
             tc.tile_pool(name="ot_ps", bufs=2, space="PSUM") as ot_ps, \
             tc.tile_pool(name="wk_ps", bufs=2, space="PSUM") as wk_ps:

            for g in range(GROUPS):
                gcol = slice(g * 512, (g + 1) * 512)
                # stream this group's decompress weights
                wdk_t = [wdkp.tile([128, 512], F32R, tag="wdk", name="wdk_t") for _ in range(NKV)]
                for c in range(NKV):
                    nc.sync.dma_start(
                        out=wdk_t[c], in_=t["wdk"][c * 128:(c + 1) * 128, gcol].bitcast(F32R))
                wdv_t = [wdvp.tile([128, 512], F32R, tag="wdv", name="wdv_t") for _ in range(NKV)]
                for c in range(NKV):
                    nc.sync.dma_start(
                        out=wdv_t[c], in_=t["wdv"][c * 128:(c + 1) * 128, gcol].bitcast(F32R))
                wdq_t = [wdqp.tile([128, 512], F32R, tag="wdq", name="wdq_t") for _ in range(NQL)]
                for c in range(NQL):
                    nc.sync.dma_start(
                        out=wdq_t[c], in_=t["wdq"][c * 128:(c + 1) * 128, gcol].bitcast(F32R))
                grcol = slice(g * 256, (g + 1) * 256)
                wdqr_t = [wdqrp.tile([128, 256], F32R, tag="wdqr", name="wdqr_t") for _ in range(NQL)]
                for c in range(NQL):
                    nc.sync.dma_start(
                        out=wdqr_t[c], in_=t["wdqr"][c * 128:(c + 1) * 128, grcol].bitcast(F32R))

                vt = {}
                qr_roped = None
                for hl in range(GH):
                    h = g * GH + hl
                    pair, hp = hl // 2, hl % 2
                    hcol = slice(hl * 128, (hl + 1) * 128)

                    # v for this pair (N=256 keeps fp32r full rate)
                    if hp == 0:
                        pcol = slice(pair * 256, (pair + 1) * 256)
                        for st in range(NST):
                            pv = wk_ps.tile([128, 256], F32, tag="wk")
                            for c in range(NKV):
                                nc.tensor.matmul(
                                    pv, _r(kvcT[c][:, st * 128:(st + 1) * 128]),
                                    _r(wdv_t[c][:, pcol]),
                                    start=(c == 0), stop=(c == NKV - 1))
                            v_s = vpool.tile([128, 256], BF16, tag="v")
                            nc.scalar.copy(v_s, pv)
                            vt[(pair, st)] = v_s

                    # k_nope^T for this head: [128 d, S]
                    kh = khp.tile([128, S], F32R, tag="kh")
                    for blk in range(NS):
                        bcol = slice(blk * 512, (blk + 1) * 512)
                        pk = wk_ps.tile([128, 512], F32, tag="wk")
                        for c in range(NKV):
                            nc.tensor.matmul(
                                pk, _r(wdk_t[c][:, hcol]), _r(kvcT[c][:, bcol]),
                                start=(c == 0), stop=(c == NKV - 1))
                        nc.scalar.copy(kh[:, bcol], pk)

                    # q_nope^T for this head: [128 d, SQ]
                    pq = wk_ps.tile([128, SQ], F32, tag="wk")
                    for c in range(NQL):
                        nc.tensor.matmul(pq, _r(wdq_t[c][:, hcol]), _r(qcT[c]),
                                         start=(c == 0), stop=(c == NQL - 1))
                    qn = qnp.tile([128, SQ], F32R, tag="qn")
                    nc.scalar.copy(qn, pq)

                    # q_rope for the pair (two heads stacked on partitions)
                    if hp == 0:
                        prcol = slice(pair * 128, (pair + 1) * 128)
                        pqr = wk_ps.tile([128, SQ], F32, tag="wk")
                        for c in range(NQL):
                            nc.tensor.matmul(
                                pqr, _r(wdqr_t[c][:, prcol]), _r(qcT[c]),
                                start=(c == 0), stop=(c == NQL - 1))
                        qraw = misc.tile([128, SQ], F32R, tag="qraw")
                        nc.scalar.copy(qraw, pqr)
                        rot = aux_ps.tile([128, SQ], F32, tag="aux")
                        nc.tensor.matmul(rot, _r(rotp_s), _r(qraw))
                        t1 = misc.tile([128, SQ], F32, tag="ropet1")
                        nc.vector.tensor_mul(t1, qraw, cosq_s)
                        t2 = misc.tile([128, SQ], F32, tag="ropet2")
                        nc.vector.tensor_mul(t2, rot, sinq_s)
                        qr_roped = qrp.tile([128, SQ], F32R, tag="qr")
                        nc.vector.tensor_add(qr_roped, t1, t2)
                    hrow = slice(hp * 64, (hp + 1) * 64)

                    # attention: S^T tiles, exp, denominator, P^T@V
                    pot = ot_ps.tile([128, SQ], F32, tag="ot")
                    den = denp.tile([128, SQ], F32R, tag="den")
                    for kt in range(NST):
                        kcol = slice(kt * 128, (kt + 1) * 128)
                        pst = st_ps.tile([128, SQ], F32, tag="st")
                        nc.tensor.matmul(pst, _r(kh[:, kcol]), _r(qn),
                                         start=True, stop=False)
                        nc.tensor.matmul(pst, _r(krT[hrow, kcol]),
                                         _r(qr_roped[hrow, :]),
                                         start=False, stop=True)
                        pt = ptp.tile([128, SQ], BF16, tag="pt")
                        nc.scalar.activation(pt, pst,
                                             mybir.ActivationFunctionType.Exp,
                                             scale=SCALE)
                        if kt == 0:
                            nc.vector.tensor_copy(den, pt)
                        else:
                            nc.vector.tensor_add(den, den, pt)
                        vs = vt[(pair, kt)]
                        nc.tensor.matmul(
                            pot, vs[:, hp * 128:(hp + 1) * 128], pt,
                            start=(kt == 0), stop=(kt == NST - 1))

                    den1 = aux_ps.tile([1, SQ], F32, tag="aux")
                    nc.tensor.matmul(den1, _r(ones128), _r(den))
                    rec = misc.tile([1, SQ], F32R, tag="rec")
                    nc.scalar.copy(rec, den1)
                    with nc.allow_low_precision(reason="f32r is full fp32 bits"):
                        nc.vector.reciprocal(rec, rec)
                    bc = aux_ps.tile([128, SQ], F32, tag="aux")
                    nc.tensor.matmul(bc, _r(ones1), _r(rec))
                    bc_s = misc.tile([128, SQ], F32, tag="bcs")
                    nc.scalar.copy(bc_s, bc)
                    nc.vector.tensor_mul(oT[h], pot, bc_s)

        if debug:
            for c in range(NKV):
                nc.sync.dma_start(
                    out=t["dbg_kvcT"][c * 128:(c + 1) * 128, :],
                    in_=kvcT[c].bitcast(F32))
            for c in range(NQL):
                nc.sync.dma_start(
                    out=t["dbg_qcT"][c * 128:(c + 1) * 128, :],
                    in_=qcT[c].bitcast(F32))
            nc.sync.dma_start(out=t["dbg_krT"][:, :], in_=krT.bitcast(F32))
            for h in range(H):
                nc.sync.dma_start(
                    out=t["dbg_oT"][h * 128:(h + 1) * 128, :],
                    in_=oT[h].bitcast(F32))

        # ===================================================================
        # Phase B: output projection, all 16 heads, PSUM-accumulated.
        # Loop order: wo tile loads once per (h, blk) and serves all 4 query
        # tiles (wo HBM traffic 16MB instead of 64MB).
        # ===================================================================
        NQT = SQ // 128
        with nc.named_scope("pB_outproj", notify=True), \
             tc.tile_pool(name="wop", bufs=4) as wop, \
             tc.tile_pool(name="outs", bufs=4) as outs, \
             tc.tile_pool(name="po_ps", bufs=4, space="PSUM") as po_ps:
            for blk in range(NS):
                bcol = slice(blk * 512, (blk + 1) * 512)
                po = [po_ps.tile([128, 512], F32, tag="po", name=f"po{qt}")
                      for qt in range(NQT)]
                for h in range(H):
                    wo_t = wop.tile([128, 512], F32R, tag="wo")
                    nc.sync.dma_start(
                        out=wo_t, in_=t["wo"][h * 128:(h + 1) * 128, bcol].bitcast(F32R))
                    for qt in range(NQT):
                        nc.tensor.matmul(
                            po[qt], _r(oT[h][:, qt * 128:(qt + 1) * 128]), _r(wo_t),
                            start=(h == 0), stop=(h == H - 1))
                for qt in range(NQT):
                    o_s = outs.tile([128, 512], F32, tag="os")
                    nc.scalar.copy(o_s, po[qt])
                    nc.sync.dma_start(
                        out=t["out"][qt * 128:(qt + 1) * 128, bcol], in_=o_s)


_NC_CACHE = None


def _get_nc():
    global _NC_CACHE
    if _NC_CACHE is None:
        _NC_CACHE = build_nc()
    return _NC_CACHE


def _rope_tables(positions):
    """cos/sin tables in transposed-packed layout [128, len(positions)]:
    rows 0:64 and 64:128 both hold the [RD, s] table (two rope vectors are
    stacked per 128 partitions)."""
    inv_freq = 1.0 / (10000.0 ** (np.arange(0, RD, 2, dtype=np.float32) / RD))
    ang = positions[:, None].astype(np.float32) * inv_freq[None, :]  # [s, 32]
    cos = np.concatenate([np.cos(ang), np.cos(ang)], axis=-1)        # [s, 64]
    sin = np.concatenate([np.sin(ang), np.sin(ang)], axis=-1)
    cosT = np.ascontiguousarray(cos.T)                               # [64, s]
    sinT = np.ascontiguousarray(sin.T)
    return (np.concatenate([cosT, cosT], axis=0),
            np.concatenate([sinT, sinT], axis=0))


def _rot_perm():
    m = np.zeros((128, 128), dtype=np.float32)
    for b0 in (0, 64):
        for i in range(32):
            m[b0 + i + 32, b0 + i] = -1.0   # rot[m] = -t[m+32], m < 32
            m[b0 + i, b0 + i + 32] = 1.0    # rot[m] = +t[m-32], m >= 32
    return m


def kernel(x, Wcq, g_q, Wdq, Wdqr, Wckv, g_kv, Wdk, Wdv, Wkr, Wo):
    nc = _get_nc()

    x = np.asarray(x, dtype=np.float32)
    xT = [np.ascontiguousarray(x[b].T) for b in range(B)]  # [D, S] each
    wkr2 = np.ascontiguousarray(
        np.concatenate([Wkr, Wkr], axis=1)).astype(np.float32)  # [D, 128]
    cosk, sink = _rope_tables(np.arange(S))
    rotp = _rot_perm()

    shared = {
        "wcq": np.ascontiguousarray(Wcq, dtype=np.float32),
        "wckv": np.ascontiguousarray(Wckv, dtype=np.float32),
        "wkr2": wkr2,
        "wdq": np.ascontiguousarray(Wdq, dtype=np.float32),
        "wdqr": np.ascontiguousarray(Wdqr, dtype=np.float32),
        "wdk": np.ascontiguousarray(Wdk, dtype=np.float32),
        "wdv": np.ascontiguousarray(Wdv, dtype=np.float32),
        "wo": np.ascontiguousarray(Wo, dtype=np.float32),
        "gq": np.ascontiguousarray(g_q, dtype=np.float32).reshape(1, QL),
        "gkv": np.ascontiguousarray(g_kv, dtype=np.float32).reshape(1, KV),
        "cosk": np.ascontiguousarray(cosk),
        "sink": np.ascontiguousarray(sink),
        "rotp": rotp,
    }

    in_maps = []
    for core in range(N_CORES):
        b, sl = core // 4, core % 4
        rows = np.arange(sl * SQ, (sl + 1) * SQ)
        cq, sq_t = _rope_tables(rows)
        m = dict(shared)
        m["xT"] = xT[b]
        m["xqT"] = np.ascontiguousarray(xT[b][:, sl * SQ:(sl + 1) * SQ])
        m["cosq"] = np.ascontiguousarray(cq)
        m["sinq"] = np.ascontiguousarray(sq_t)
        in_maps.append(m)

    trace = bool(int(os.environ.get("MLA_TRACE", "0")))
    res = run_bass_kernel_spmd(
        nc, in_maps, core_ids=list(range(N_CORES)), trace=trace,
        trace_cores=list(range(N_CORES)) if trace else None,
        stitch_traces=bool(int(os.environ.get("MLA_STITCH", "0"))),
        tmpdir=os.environ.get("MLA_TMPDIR") or None,
    )
    kernel.last_result = res

    out = np.empty((B, S, D), dtype=np.float32)
    for core in range(N_CORES):
        b, sl = core // 4, core % 4
        out[b, sl * SQ:(sl + 1) * SQ, :] = res.results[core]["out"]
    return out



# revision 19
# speedup vs baseline: 1.5300x; 1.5300x over previous
"""MLA forward Bass kernel for 8 TRN2 NeuronCores.

Sharding: pure query-row sharding. Core c handles batch b = c//4 and query rows
[sl*512, (sl+1)*512) with sl = c%4, for ALL 16 heads. Keys/values span the full
sequence, so the compressed-KV path (kvc, k_rope) is computed per-core for the
whole batch (replicated across the 4 cores that share a batch), while the Q
path, attention, and the output projection only cover the core's 512 query
rows. The full output-projection contraction (all 16 heads) is local, so no
cross-core reduction is needed: the host just concatenates the 8 row-blocks.

Layouts: everything TensorE-facing is kept transposed ([feature, seq]) so the
feature dim sits on partitions and matmuls contract over it. Softmax runs on
S^T tiles [k, q]: exp on ACT (no max-shift; scores are O(1) here), denominator
via DVE tile-adds + a ones-matmul partition reduction, normalization folded
into the PSUM->SBUF drain of the attention output. RoPE's rotate-half is a
constant 128x128 permutation matmul. All matmuls run as float32r.
"""

import os
import sys

for _p in ("/root/.axon_site/_ro/trn_rl_repo", "/opt/trn_rl_repo"):
    if os.path.isdir(_p) and _p not in sys.path:
        sys.path.insert(0, _p)

import numpy as np

import concourse.bass as bass
import concourse.tile as tile
from concourse import mybir
from concourse.bass_utils import run_bass_kernel_spmd

F32 = mybir.dt.float32
F32R = mybir.dt.float32r
BF16 = mybir.dt.bfloat16

D = 2048        # d_model
S = 2048        # seq len
B = 2           # batch
H = 16          # heads
HD = 128        # nope head dim
KV = 512        # kv lora rank
QL = 768        # q lora rank
RD = 64         # rope dim
EPS = 1e-6
SQ = 512        # query rows per core
N_CORES = 8
GROUPS = 4      # head groups of 4
GH = 4          # heads per group
SCALE = 1.0 / float(np.sqrt(HD + RD))

NKV = KV // 128   # 4 kv-lora chunks
NQL = QL // 128   # 6 q-lora chunks
NS = S // 512     # 4 seq blocks
NST = S // 128    # 16 seq tiles


# ---------------------------------------------------------------------------
# The walrus build in this container only encodes a single sync-wait on a
# Drain (TPB_CTRL) instruction, but TileContext._drain_and_barrier parks the
# whole global-clock wait set on the tail drain ("Too many sync wait
# commands"). Hoist the waits onto single-wait NOPs ahead of a bare drain.
def _patch_tile_drain():
    from bass_rust import ScopedClock

    def _drain_and_barrier(self, tick_clock, wait_clock):
        probe = self.nc.sync.nop(nofuse=True)
        wait_clock.add_sem_waits(
            probe.ins, ScopedClock({None: tick_clock.global_clock})
        )
        si = probe.ins.sync_info
        waits = list(si.on_wait) if si is not None else []
        if len(waits) > 1:
            probe.ins.sync_info = mybir.SyncInfo(on_wait=waits[:1], on_update=[])
            for w in waits[1:]:
                extra = self.nc.sync.nop(nofuse=True)
                extra.ins.sync_info = mybir.SyncInfo(on_wait=[w], on_update=[])
        self.nc.sync.drain()

        self.nc.all_engine_barrier()
        assert self.sems is not None
        popped = self.nc._tile_sem_poison_stack.pop()
        assert popped is self._sem_poison
        self.nc.clear_and_free_semaphores(list(self.sems.allocated().values()))
        self.nc.all_engine_barrier()

    tile.TileContext._drain_and_barrier = _drain_and_barrier


_patch_tile_drain()


def _r(ap):
    return ap.bitcast(F32R)


def build_nc():
    nc = bass.Bass()

    xT = nc.dram_tensor("xT", [D, S], BF16, kind="ExternalInput")
    xqT = nc.dram_tensor("xqT", [D, SQ], BF16, kind="ExternalInput")
    wcq = nc.dram_tensor("wcq", [D, QL], BF16, kind="ExternalInput")
    wckv = nc.dram_tensor("wckv", [D, KV], BF16, kind="ExternalInput")
    wkr2 = nc.dram_tensor("wkr2", [D, 128], BF16, kind="ExternalInput")
    wdq = nc.dram_tensor("wdq", [QL, H * HD], BF16, kind="ExternalInput")
    wdqr = nc.dram_tensor("wdqr", [QL, H * RD], BF16, kind="ExternalInput")
    wdk = nc.dram_tensor("wdk", [KV, H * HD], BF16, kind="ExternalInput")
    wdv = nc.dram_tensor("wdv", [KV, H * HD], BF16, kind="ExternalInput")
    wo = nc.dram_tensor("wo", [H * HD, D], BF16, kind="ExternalInput")
    gq = nc.dram_tensor("gq", [1, QL], F32, kind="ExternalInput")
    gkv = nc.dram_tensor("gkv", [1, KV], F32, kind="ExternalInput")
    cosk = nc.dram_tensor("cosk", [128, S], F32, kind="ExternalInput")
    sink = nc.dram_tensor("sink", [128, S], F32, kind="ExternalInput")
    cosq = nc.dram_tensor("cosq", [128, SQ], F32, kind="ExternalInput")
    sinq = nc.dram_tensor("sinq", [128, SQ], F32, kind="ExternalInput")
    rotp = nc.dram_tensor("rotp", [128, 128], BF16, kind="ExternalInput")
    out = nc.dram_tensor("out", [SQ, D], F32, kind="ExternalOutput")
    debug = bool(int(os.environ.get("MLA_DEBUG", "0")))
    if debug:
        dbg_kvcT = nc.dram_tensor("dbg_kvcT", [KV, S], BF16, kind="ExternalOutput")
        dbg_qcT = nc.dram_tensor("dbg_qcT", [QL, SQ], BF16, kind="ExternalOutput")
        dbg_krT = nc.dram_tensor("dbg_krT", [128, S], BF16, kind="ExternalOutput")
        dbg_oT = nc.dram_tensor("dbg_oT", [H * 128, SQ], BF16, kind="ExternalOutput")

    with tile.TileContext(nc) as tc:
        _build_body(nc, tc, locals(), debug)
    _split_excess_waits(nc)
    return nc


# This walrus build encodes at most one sync-wait per engine instruction;
# hoist surplus waits onto single-wait NOPs right before the instruction on
# the same engine queue (in-order execution keeps the semantics identical).
def _split_excess_waits(nc, max_waits=1):
    n_nops = 0
    for f in nc.m.functions:
        for bb in f.blocks:
            out = []
            for ins in bb.instructions:
                si = ins.sync_info
                if si is not None:
                    sem = [w for w in si.on_wait if w.sync_type == "semaphore"]
                    other = [w for w in si.on_wait if w.sync_type != "semaphore"]
                    budget = max(max_waits - len(other), 0)
                    if len(sem) > budget:
                        extra, keep = sem[:-budget] if budget else sem, (
                            sem[-budget:] if budget else [])
                        for j, w in enumerate(extra):
                            nop = mybir.InstNoOp(
                                name=f"{ins.name}-wsplit{j}",
                                engine=ins.engine,
                                bass_nofuse=True,
                                sync_info=mybir.SyncInfo(
                                    on_wait=[w], on_update=[]),
                            )
                            out.append(nop)
                            n_nops += 1
                        ins.sync_info = mybir.SyncInfo(
                            on_wait=other + keep,
                            on_update=list(si.on_update))
                out.append(ins)
            bb.instructions = out
    return n_nops


def _build_body(nc, tc, t, debug=False):
    from contextlib import ExitStack

    ctx = ExitStack()
    with ctx:
        consts = ctx.enter_context(tc.tile_pool(name="consts", bufs=1))
        persist = ctx.enter_context(tc.tile_pool(name="persist", bufs=1))
        misc = ctx.enter_context(tc.tile_pool(name="misc", bufs=2))
        # PSUM pools: aux first (lives through whole kernel), then phase pools.
        aux_ps = ctx.enter_context(tc.tile_pool(name="aux_ps", bufs=2, space="PSUM"))

        # ---- constants -----------------------------------------------------
        ones128f = consts.tile([128, 1], F32)
        nc.vector.memset(ones128f, 1.0)
        ones128 = consts.tile([128, 1], F32R)
        nc.scalar.copy(ones128, ones128f)
        ones1f = consts.tile([1, 128], F32)
        nc.vector.memset(ones1f, 1.0)
        ones1 = consts.tile([1, 128], F32R)
        nc.scalar.copy(ones1, ones1f)
        gq_s = consts.tile([1, QL], F32R)
        nc.sync.dma_start(out=gq_s, in_=t["gq"][:, :].bitcast(F32R))
        gkv_s = consts.tile([1, KV], F32R)
        nc.sync.dma_start(out=gkv_s, in_=t["gkv"][:, :].bitcast(F32R))
        eps_s = consts.tile([1, 1], F32)
        nc.vector.memset(eps_s, EPS)
        rotp_s = consts.tile([128, 128], BF16)
        nc.sync.dma_start(out=rotp_s, in_=t["rotp"][:, :])
        cosq_s = consts.tile([128, SQ], F32)
        nc.sync.dma_start(out=cosq_s, in_=t["cosq"][:, :])
        sinq_s = consts.tile([128, SQ], F32)
        nc.sync.dma_start(out=sinq_s, in_=t["sinq"][:, :])

        # ---- persistent tiles (bf16: matmul operands -> FWL weight loads) --
        kvcT = [persist.tile([128, S], BF16, tag=f"kvcT{c}", name=f"kvcT{c}") for c in range(NKV)]
        krT = persist.tile([128, S], BF16, tag="krT")
        qcT = [persist.tile([128, SQ], BF16, tag=f"qcT{c}", name=f"qcT{c}") for c in range(NQL)]
        oT = [persist.tile([128, SQ], BF16, tag=f"oT{h}", name=f"oT{h}") for h in range(H)]

        # ===================================================================
        # Phase 0: compress. kvcT/krT over full seq, qcT over own query rows.
        # ===================================================================
        with nc.named_scope("p0_compress", notify=True), \
             tc.tile_pool(name="misc0", bufs=2) as misc0, \
             tc.tile_pool(name="xtp", bufs=3) as xtp, \
             tc.tile_pool(name="wkvhold", bufs=1) as wkvhold, \
             tc.tile_pool(name="wstream", bufs=3) as wstream, \
             tc.tile_pool(name="acc_ps", bufs=6, space="PSUM") as acc_ps:
            # wckv/wkr are reused by all 4 seq blocks: load once, keep in SBUF
            wkv_h = [wkvhold.tile([128, KV], BF16, tag=f"wckv{d}", name=f"wckv{d}")
                     for d in range(16)]
            wkr_h = [wkvhold.tile([128, 128], BF16, tag=f"wkr{d}", name=f"wkr{d}")
                     for d in range(16)]
            for d in range(16):
                drow = slice(d * 128, (d + 1) * 128)
                nc.sync.dma_start(out=wkv_h[d], in_=t["wckv"][drow, :])
                nc.sync.dma_start(out=wkr_h[d], in_=t["wkr2"][drow, :])
            for sb in range(NS):
                scol = slice(sb * 512, (sb + 1) * 512)
                pkv = [acc_ps.tile([128, 512], F32, tag="acc", name="pkv") for _ in range(NKV)]
                pkr = acc_ps.tile([128, 512], F32, tag="acc")
                for d in range(16):
                    drow = slice(d * 128, (d + 1) * 128)
                    xt = xtp.tile([128, 512], BF16, tag="xt")
                    nc.sync.dma_start(out=xt, in_=t["xT"][drow, scol])
                    for c in range(NKV):
                        nc.tensor.matmul(
                            pkv[c], wkv_h[d][:, c * 128:(c + 1) * 128], xt,
                            start=(d == 0), stop=(d == 15))
                    nc.tensor.matmul(pkr, wkr_h[d], xt,
                                     start=(d == 0), stop=(d == 15))

                # rmsnorm over kv features (partition dim across the 4 chunks)
                ssq = aux_ps.tile([1, 512], F32, tag="aux")
                for c in range(NKV):
                    sq = misc0.tile([128, 512], F32R, tag="sq")
                    nc.scalar.square(sq, pkv[c])
                    nc.tensor.matmul(ssq, _r(ones128), _r(sq),
                                     start=(c == 0), stop=(c == NKV - 1))
                rstd = misc0.tile([1, 512], F32R, tag="rstd")
                nc.scalar.activation(rstd, ssq,
                                     mybir.ActivationFunctionType.Sqrt,
                                     bias=eps_s[:, :], scale=1.0 / KV)
                with nc.allow_low_precision(reason="f32r is full fp32 bits"):
                    nc.vector.reciprocal(rstd, rstd)
                for c in range(NKV):
                    bc = aux_ps.tile([128, 512], F32, tag="aux")
                    nc.tensor.matmul(
                        bc, _r(gkv_s[:, c * 128:(c + 1) * 128]), _r(rstd))
                    bc_s = misc.tile([128, 512], F32, tag="bcs")
                    nc.scalar.copy(bc_s, bc)
                    nc.vector.tensor_mul(kvcT[c][:, scol], pkv[c], bc_s)

                # rope on the (duplicated-rows) k_rope block
                ck = misc0.tile([128, 512], F32, tag="ck")
                nc.sync.dma_start(out=ck, in_=t["cosk"][:, scol])
                sk = misc0.tile([128, 512], F32, tag="sk")
                nc.sync.dma_start(out=sk, in_=t["sink"][:, scol])
                kraw = misc0.tile([128, 512], BF16, tag="kraw")
                nc.scalar.copy(kraw, pkr)
                rot = aux_ps.tile([128, 512], F32, tag="aux")
                nc.tensor.matmul(rot, rotp_s, kraw)
                t1 = misc0.tile([128, 512], F32, tag="ropet1")
                nc.vector.tensor_mul(t1, kraw, ck)
                t2 = misc0.tile([128, 512], F32, tag="ropet2")
                nc.vector.tensor_mul(t2, rot, sk)
                nc.vector.tensor_add(krT[:, scol], t1, t2)

            # qcT over own query rows
            pqc = [acc_ps.tile([128, 512], F32, tag="acc", name="pqc") for _ in range(NQL)]
            for d in range(16):
                drow = slice(d * 128, (d + 1) * 128)
                xt = xtp.tile([128, 512], BF16, tag="xt")
                nc.sync.dma_start(out=xt, in_=t["xqT"][drow, :])
                wq_t = wstream.tile([128, QL], BF16, tag="wcq")
                nc.sync.dma_start(out=wq_t, in_=t["wcq"][drow, :])
                for c in range(NQL):
                    nc.tensor.matmul(
                        pqc[c], wq_t[:, c * 128:(c + 1) * 128], xt,
                        start=(d == 0), stop=(d == 15))
            ssq = aux_ps.tile([1, 512], F32, tag="aux")
            for c in range(NQL):
                sq = misc0.tile([128, 512], F32R, tag="sq")
                nc.scalar.square(sq, pqc[c])
                nc.tensor.matmul(ssq, _r(ones128), _r(sq),
                                 start=(c == 0), stop=(c == NQL - 1))
            rstd = misc0.tile([1, 512], F32R, tag="rstd")
            nc.scalar.activation(rstd, ssq, mybir.ActivationFunctionType.Sqrt,
                                 bias=eps_s[:, :], scale=1.0 / QL)
            with nc.allow_low_precision(reason="f32r is full fp32 bits"):
                nc.vector.reciprocal(rstd, rstd)
            for c in range(NQL):
                bc = aux_ps.tile([128, 512], F32, tag="aux")
                nc.tensor.matmul(bc, _r(gq_s[:, c * 128:(c + 1) * 128]), _r(rstd))
                bc_s = misc.tile([128, 512], F32, tag="bcs")
                nc.scalar.copy(bc_s, bc)
                nc.vector.tensor_mul(qcT[c], pqc[c], bc_s)

        # ===================================================================
        # Phase A: per head group -- decompress k/v/q, attention.
        # ===================================================================
        with nc.named_scope("pA_attn", notify=True), \
             tc.tile_pool(name="vpool", bufs=24) as vpool, \
             tc.tile_pool(name="khp", bufs=2) as khp, \
             tc.tile_pool(name="qnp", bufs=2) as qnp, \
             tc.tile_pool(name="qrp", bufs=2) as qrp, \
             tc.tile_pool(name="ptp", bufs=4) as ptp, \
             tc.tile_pool(name="denp", bufs=2) as denp, \
             tc.tile_pool(name="wdqp", bufs=6) as wdqp, \
             tc.tile_pool(name="wdqrp", bufs=6) as wdqrp, \
             tc.tile_pool(name="wdkp", bufs=4) as wdkp, \
             tc.tile_pool(name="wdvp", bufs=4) as wdvp, \
             tc.tile_pool(name="st_ps", bufs=2, space="PSUM") as st_ps, \
             tc.tile_pool(name="ot_ps", bufs=2, space="PSUM") as ot_ps, \
             tc.tile_pool(name="wk_ps", bufs=2, space="PSUM") as wk_ps:

            for g in range(GROUPS):
                gcol = slice(g * 512, (g + 1) * 512)
                # stream this group's decompress weights
                wdk_t = [wdkp.tile([128, 512], BF16, tag="wdk", name="wdk_t") for _ in range(NKV)]
                for c in range(NKV):
                    nc.sync.dma_start(
                        out=wdk_t[c], in_=t["wdk"][c * 128:(c + 1) * 128, gcol])
                wdv_t = [wdvp.tile([128, 512], BF16, tag="wdv", name="wdv_t") for _ in range(NKV)]
                for c in range(NKV):
                    nc.sync.dma_start(
                        out=wdv_t[c], in_=t["wdv"][c * 128:(c + 1) * 128, gcol])
                wdq_t = [wdqp.tile([128, 512], BF16, tag="wdq", name="wdq_t") for _ in range(NQL)]
                for c in range(NQL):
                    nc.sync.dma_start(
                        out=wdq_t[c], in_=t["wdq"][c * 128:(c + 1) * 128, gcol])
                grcol = slice(g * 256, (g + 1) * 256)
                wdqr_t = [wdqrp.tile([128, 256], BF16, tag="wdqr", name="wdqr_t") for _ in range(NQL)]
                for c in range(NQL):
                    nc.sync.dma_start(
                        out=wdqr_t[c], in_=t["wdqr"][c * 128:(c + 1) * 128, grcol])

                # v for all 4 heads of the group: moving = wdv (512 wide),
                # stationary = kvc seq-tile. Halves the matmul/LDW count vs
                # the per-pair 256-wide variant.
                vt = {}
                for st in range(NST):
                    pv = wk_ps.tile([128, 512], F32, tag="wk")
                    for c in range(NKV):
                        nc.tensor.matmul(
                            pv, kvcT[c][:, st * 128:(st + 1) * 128], wdv_t[c],
                            start=(c == 0), stop=(c == NKV - 1))
                    v_s = vpool.tile([128, 512], BF16, tag="v")
                    nc.scalar.copy(v_s, pv)
                    vt[st] = v_s

                qr_roped = None
                for hl in range(GH):
                    h = g * GH + hl
                    pair, hp = hl // 2, hl % 2
                    hcol = slice(hl * 128, (hl + 1) * 128)

                    # k_nope^T for this head: [128 d, S]
                    kh = khp.tile([128, S], BF16, tag="kh")
                    for blk in range(NS):
                        bcol = slice(blk * 512, (blk + 1) * 512)
                        pk = wk_ps.tile([128, 512], F32, tag="wk")
                        for c in range(NKV):
                            nc.tensor.matmul(
                                pk, wdk_t[c][:, hcol], kvcT[c][:, bcol],
                                start=(c == 0), stop=(c == NKV - 1))
                        nc.scalar.copy(kh[:, bcol], pk)

                    # q_nope^T for this head: [128 d, SQ]
                    pq = wk_ps.tile([128, SQ], F32, tag="wk")
                    for c in range(NQL):
                        nc.tensor.matmul(pq, wdq_t[c][:, hcol], qcT[c],
                                         start=(c == 0), stop=(c == NQL - 1))
                    qn = qnp.tile([128, SQ], BF16, tag="qn")
                    nc.scalar.copy(qn, pq)

                    # q_rope for the pair (two heads stacked on partitions)
                    if hp == 0:
                        prcol = slice(pair * 128, (pair + 1) * 128)
                        pqr = wk_ps.tile([128, SQ], F32, tag="wk")
                        for c in range(NQL):
                            nc.tensor.matmul(
                                pqr, wdqr_t[c][:, prcol], qcT[c],
                                start=(c == 0), stop=(c == NQL - 1))
                        qraw = misc.tile([128, SQ], BF16, tag="qraw")
                        nc.scalar.copy(qraw, pqr)
                        rot = aux_ps.tile([128, SQ], F32, tag="aux")
                        nc.tensor.matmul(rot, rotp_s, qraw)
                        t1 = misc.tile([128, SQ], F32, tag="ropet1")
                        nc.vector.tensor_mul(t1, qraw, cosq_s)
                        t2 = misc.tile([128, SQ], F32, tag="ropet2")
                        nc.vector.tensor_mul(t2, rot, sinq_s)
                        qr_roped = qrp.tile([128, SQ], BF16, tag="qr")
                        nc.vector.tensor_add(qr_roped, t1, t2)
                    hrow = slice(hp * 64, (hp + 1) * 64)

                    # attention: S^T tiles, exp, denominator, P^T@V
                    pot = ot_ps.tile([128, SQ], F32, tag="ot")
                    den = denp.tile([128, SQ], F32R, tag="den")
                    for kt in range(NST):
                        kcol = slice(kt * 128, (kt + 1) * 128)
                        pst = st_ps.tile([128, SQ], F32, tag="st")
                        nc.tensor.matmul(pst, kh[:, kcol], qn,
                                         start=True, stop=False)
                        nc.tensor.matmul(pst, krT[hrow, kcol],
                                         qr_roped[hrow, :],
                                         start=False, stop=True)
                        pt = ptp.tile([128, SQ], BF16, tag="pt")
                        nc.scalar.activation(pt, pst,
                                             mybir.ActivationFunctionType.Exp,
                                             scale=SCALE)
                        if kt == 0:
                            nc.vector.tensor_copy(den, pt)
                        else:
                            nc.vector.tensor_add(den, den, pt)
                        vs = vt[kt]
                        nc.tensor.matmul(
                            pot, vs[:, hl * 128:(hl + 1) * 128], pt,
                            start=(kt == 0), stop=(kt == NST - 1))

                    den1 = aux_ps.tile([1, SQ], F32, tag="aux")
                    nc.tensor.matmul(den1, _r(ones128), _r(den))
                    rec = misc.tile([1, SQ], F32R, tag="rec")
                    nc.scalar.copy(rec, den1)
                    with nc.allow_low_precision(reason="f32r is full fp32 bits"):
                        nc.vector.reciprocal(rec, rec)
                    bc = aux_ps.tile([128, SQ], F32, tag="aux")
                    nc.tensor.matmul(bc, _r(ones1), _r(rec))
                    bc_s = misc.tile([128, SQ], F32, tag="bcs")
                    nc.scalar.copy(bc_s, bc)
                    nc.vector.tensor_mul(oT[h], pot, bc_s)

        if debug:
            for c in range(NKV):
                nc.sync.dma_start(
                    out=t["dbg_kvcT"][c * 128:(c + 1) * 128, :], in_=kvcT[c])
            for c in range(NQL):
                nc.sync.dma_start(
                    out=t["dbg_qcT"][c * 128:(c + 1) * 128, :], in_=qcT[c])
            nc.sync.dma_start(out=t["dbg_krT"][:, :], in_=krT)
            for h in range(H):
                nc.sync.dma_start(
                    out=t["dbg_oT"][h * 128:(h + 1) * 128, :], in_=oT[h])

        # ===================================================================
        # Phase B: output projection, all 16 heads, PSUM-accumulated.
        # Loop order: wo tile loads once per (h, blk) and serves all 4 query
        # tiles (wo HBM traffic 16MB instead of 64MB).
        # ===================================================================
        NQT = SQ // 128
        with nc.named_scope("pB_outproj", notify=True), \
             tc.tile_pool(name="wop", bufs=4) as wop, \
             tc.tile_pool(name="outs", bufs=4) as outs, \
             tc.tile_pool(name="po_ps", bufs=4, space="PSUM") as po_ps:
            for blk in range(NS):
                bcol = slice(blk * 512, (blk + 1) * 512)
                po = [po_ps.tile([128, 512], F32, tag="po", name=f"po{qt}")
                      for qt in range(NQT)]
                for h in range(H):
                    wo_t = wop.tile([128, 512], BF16, tag="wo")
                    nc.sync.dma_start(
                        out=wo_t, in_=t["wo"][h * 128:(h + 1) * 128, bcol])
                    for qt in range(NQT):
                        nc.tensor.matmul(
                            po[qt], oT[h][:, qt * 128:(qt + 1) * 128], wo_t,
                            start=(h == 0), stop=(h == H - 1))
                for qt in range(NQT):
                    o_s = outs.tile([128, 512], F32, tag="os")
                    nc.scalar.copy(o_s, po[qt])
                    nc.sync.dma_start(
                        out=t["out"][qt * 128:(qt + 1) * 128, bcol], in_=o_s)


_NC_CACHE = None


def _get_nc():
    global _NC_CACHE
    if _NC_CACHE is None:
        _NC_CACHE = build_nc()
    return _NC_CACHE


def _rope_tables(positions):
    """cos/sin tables in transposed-packed layout [128, len(positions)]:
    rows 0:64 and 64:128 both hold the [RD, s] table (two rope vectors are
    stacked per 128 partitions)."""
    inv_freq = 1.0 / (10000.0 ** (np.arange(0, RD, 2, dtype=np.float32) / RD))
    ang = positions[:, None].astype(np.float32) * inv_freq[None, :]  # [s, 32]
    cos = np.concatenate([np.cos(ang), np.cos(ang)], axis=-1)        # [s, 64]
    sin = np.concatenate([np.sin(ang), np.sin(ang)], axis=-1)
    cosT = np.ascontiguousarray(cos.T)                               # [64, s]
    sinT = np.ascontiguousarray(sin.T)
    return (np.concatenate([cosT, cosT], axis=0),
            np.concatenate([sinT, sinT], axis=0))


def _rot_perm():
    m = np.zeros((128, 128), dtype=np.float32)
    for b0 in (0, 64):
        for i in range(32):
            m[b0 + i + 32, b0 + i] = -1.0   # rot[m] = -t[m+32], m < 32
            m[b0 + i, b0 + i + 32] = 1.0    # rot[m] = +t[m-32], m >= 32
    return m


def kernel(x, Wcq, g_q, Wdq, Wdqr, Wckv, g_kv, Wdk, Wdv, Wkr, Wo):
    import ml_dtypes

    bf16 = ml_dtypes.bfloat16
    nc = _get_nc()

    x = np.asarray(x, dtype=np.float32)
    xT = [np.ascontiguousarray(x[b].T).astype(bf16) for b in range(B)]  # [D, S]
    wkr2 = np.ascontiguousarray(
        np.concatenate([Wkr, Wkr], axis=1)).astype(bf16)  # [D, 128]
    cosk, sink = _rope_tables(np.arange(S))
    rotp = _rot_perm().astype(bf16)

    shared = {
        "wcq": np.ascontiguousarray(Wcq).astype(bf16),
        "wckv": np.ascontiguousarray(Wckv).astype(bf16),
        "wkr2": wkr2,
        "wdq": np.ascontiguousarray(Wdq).astype(bf16),
        "wdqr": np.ascontiguousarray(Wdqr).astype(bf16),
        "wdk": np.ascontiguousarray(Wdk).astype(bf16),
        "wdv": np.ascontiguousarray(Wdv).astype(bf16),
        "wo": np.ascontiguousarray(Wo).astype(bf16),
        "gq": np.ascontiguousarray(g_q, dtype=np.float32).reshape(1, QL),
        "gkv": np.ascontiguousarray(g_kv, dtype=np.float32).reshape(1, KV),
        "cosk": np.ascontiguousarray(cosk),
        "sink": np.ascontiguousarray(sink),
        "rotp": rotp,
    }

    in_maps = []
    for core in range(N_CORES):
        b, sl = core // 4, core % 4
        rows = np.arange(sl * SQ, (sl + 1) * SQ)
        cq, sq_t = _rope_tables(rows)
        m = dict(shared)
        m["xT"] = xT[b]
        m["xqT"] = np.ascontiguousarray(xT[b][:, sl * SQ:(sl + 1) * SQ])
        m["cosq"] = np.ascontiguousarray(cq)
        m["sinq"] = np.ascontiguousarray(sq_t)
        in_maps.append(m)

    trace = bool(int(os.environ.get("MLA_TRACE", "0")))
    res = run_bass_kernel_spmd(
        nc, in_maps, core_ids=list(range(N_CORES)), trace=trace,
        trace_cores=list(range(N_CORES)) if trace else None,
        stitch_traces=bool(int(os.environ.get("MLA_STITCH", "0"))),
        tmpdir=os.environ.get("MLA_TMPDIR") or None,
    )
    kernel.last_result = res

    out = np.empty((B, S, D), dtype=np.float32)
    for core in range(N_CORES):
        b, sl = core // 4, core % 4
        out[b, sl * SQ:(sl + 1) * SQ, :] = res.results[core]["out"]
    return out



# revision 27
# speedup vs baseline: 1.5360x; 1.0039x over previous
"""MLA forward Bass kernel for 8 TRN2 NeuronCores.

Sharding: pure query-row sharding. Core c handles batch b = c//4 and query rows
[sl*512, (sl+1)*512) with sl = c%4, for ALL 16 heads. Keys/values span the full
sequence, so the compressed-KV path (kvc, k_rope) is computed per-core for the
whole batch (replicated across the 4 cores that share a batch), while the Q
path, attention, and the output projection only cover the core's 512 query
rows. The full output-projection contraction (all 16 heads) is local, so no
cross-core reduction is needed: the host just concatenates the 8 row-blocks.

Layouts: everything TensorE-facing is kept transposed ([feature, seq]) so the
feature dim sits on partitions and matmuls contract over it. Softmax runs on
S^T tiles [k, q]: exp on ACT (no max-shift; scores are O(1) here), denominator
via DVE tile-adds + a ones-matmul partition reduction, normalization folded
into the PSUM->SBUF drain of the attention output. RoPE's rotate-half is a
constant 128x128 permutation matmul. All matmuls run as float32r.
"""

import os
import sys

for _p in ("/root/.axon_site/_ro/trn_rl_repo", "/opt/trn_rl_repo"):
    if os.path.isdir(_p) and _p not in sys.path:
        sys.path.insert(0, _p)

import numpy as np

import concourse.bass as bass
import concourse.tile as tile
from concourse import mybir
from concourse.bass_utils import run_bass_kernel_spmd

F32 = mybir.dt.float32
F32R = mybir.dt.float32r
BF16 = mybir.dt.bfloat16

D = 2048        # d_model
S = 2048        # seq len
B = 2           # batch
H = 16          # heads
HD = 128        # nope head dim
KV = 512        # kv lora rank
QL = 768        # q lora rank
RD = 64         # rope dim
EPS = 1e-6
SQ = 512        # query rows per core
N_CORES = 8
GROUPS = 4      # head groups of 4
GH = 4          # heads per group
SCALE = 1.0 / float(np.sqrt(HD + RD))

NKV = KV // 128   # 4 kv-lora chunks
NQL = QL // 128   # 6 q-lora chunks
NS = S // 512     # 4 seq blocks
NST = S // 128    # 16 seq tiles


# ---------------------------------------------------------------------------
# The walrus build in this container only encodes a single sync-wait on a
# Drain (TPB_CTRL) instruction, but TileContext._drain_and_barrier parks the
# whole global-clock wait set on the tail drain ("Too many sync wait
# commands"). Hoist the waits onto single-wait NOPs ahead of a bare drain.
def _patch_tile_drain():
    from bass_rust import ScopedClock

    def _drain_and_barrier(self, tick_clock, wait_clock):
        probe = self.nc.sync.nop(nofuse=True)
        wait_clock.add_sem_waits(
            probe.ins, ScopedClock({None: tick_clock.global_clock})
        )
        si = probe.ins.sync_info
        waits = list(si.on_wait) if si is not None else []
        if len(waits) > 1:
            probe.ins.sync_info = mybir.SyncInfo(on_wait=waits[:1], on_update=[])
            for w in waits[1:]:
                extra = self.nc.sync.nop(nofuse=True)
                extra.ins.sync_info = mybir.SyncInfo(on_wait=[w], on_update=[])
        self.nc.sync.drain()

        self.nc.all_engine_barrier()
        assert self.sems is not None
        popped = self.nc._tile_sem_poison_stack.pop()
        assert popped is self._sem_poison
        self.nc.clear_and_free_semaphores(list(self.sems.allocated().values()))
        self.nc.all_engine_barrier()

    tile.TileContext._drain_and_barrier = _drain_and_barrier


_patch_tile_drain()


def _r(ap):
    return ap.bitcast(F32R)


def build_nc():
    nc = bass.Bass()

    xT = nc.dram_tensor("xT", [D, S], BF16, kind="ExternalInput")
    xqT = nc.dram_tensor("xqT", [D, SQ], BF16, kind="ExternalInput")
    wcq = nc.dram_tensor("wcq", [D, QL], BF16, kind="ExternalInput")
    wckv = nc.dram_tensor("wckv", [D, KV], BF16, kind="ExternalInput")
    wkr2 = nc.dram_tensor("wkr2", [D, 128], BF16, kind="ExternalInput")
    wdq = nc.dram_tensor("wdq", [QL, H * HD], BF16, kind="ExternalInput")
    wdqr = nc.dram_tensor("wdqr", [QL, H * RD], BF16, kind="ExternalInput")
    wdk = nc.dram_tensor("wdk", [KV, H * HD], BF16, kind="ExternalInput")
    wdv = nc.dram_tensor("wdv", [KV, H * HD], BF16, kind="ExternalInput")
    wo = nc.dram_tensor("wo", [H * HD, D], BF16, kind="ExternalInput")
    gq = nc.dram_tensor("gq", [1, QL], F32, kind="ExternalInput")
    gkv = nc.dram_tensor("gkv", [1, KV], F32, kind="ExternalInput")
    cosk = nc.dram_tensor("cosk", [128, S], F32, kind="ExternalInput")
    sink = nc.dram_tensor("sink", [128, S], F32, kind="ExternalInput")
    cosq = nc.dram_tensor("cosq", [128, SQ], F32, kind="ExternalInput")
    sinq = nc.dram_tensor("sinq", [128, SQ], F32, kind="ExternalInput")
    rotp = nc.dram_tensor("rotp", [128, 128], BF16, kind="ExternalInput")
    out = nc.dram_tensor("out", [SQ, D], F32, kind="ExternalOutput")
    debug = bool(int(os.environ.get("MLA_DEBUG", "0")))
    if debug:
        dbg_kvcT = nc.dram_tensor("dbg_kvcT", [KV, S], BF16, kind="ExternalOutput")
        dbg_qcT = nc.dram_tensor("dbg_qcT", [QL, SQ], BF16, kind="ExternalOutput")
        dbg_krT = nc.dram_tensor("dbg_krT", [128, S], BF16, kind="ExternalOutput")
        dbg_oT = nc.dram_tensor("dbg_oT", [H * 128, SQ], BF16, kind="ExternalOutput")

    with tile.TileContext(nc) as tc:
        _build_body(nc, tc, locals(), debug)
    _split_excess_waits(nc)
    return nc


# This walrus build encodes at most one sync-wait per engine instruction;
# hoist surplus waits onto single-wait NOPs right before the instruction on
# the same engine queue (in-order execution keeps the semantics identical).
def _split_excess_waits(nc, max_waits=1):
    n_nops = 0
    for f in nc.m.functions:
        for bb in f.blocks:
            out = []
            for ins in bb.instructions:
                si = ins.sync_info
                if si is not None:
                    sem = [w for w in si.on_wait if w.sync_type == "semaphore"]
                    other = [w for w in si.on_wait if w.sync_type != "semaphore"]
                    budget = max(max_waits - len(other), 0)
                    if len(sem) > budget:
                        extra, keep = sem[:-budget] if budget else sem, (
                            sem[-budget:] if budget else [])
                        for j, w in enumerate(extra):
                            nop = mybir.InstNoOp(
                                name=f"{ins.name}-wsplit{j}",
                                engine=ins.engine,
                                bass_nofuse=True,
                                sync_info=mybir.SyncInfo(
                                    on_wait=[w], on_update=[]),
                            )
                            out.append(nop)
                            n_nops += 1
                        ins.sync_info = mybir.SyncInfo(
                            on_wait=other + keep,
                            on_update=list(si.on_update))
                out.append(ins)
            bb.instructions = out
    return n_nops


def _norm_flush(nc, misc, aux_ps, oT, ones128, ones1, pending):
    """Softmax normalization for the stashed (head, pot, den) tuples:
    den1 = sum_partitions(den); oT[h] = pot * (1/den1) broadcast."""
    for h, pot, den in pending:
        den1 = aux_ps.tile([1, SQ], F32, tag="aux")
        nc.tensor.matmul(den1, _r(ones128), _r(den))
        rec = misc.tile([1, SQ], F32R, tag="rec")
        nc.scalar.copy(rec, den1)
        with nc.allow_low_precision(reason="f32r is full fp32 bits"):
            nc.vector.reciprocal(rec, rec)
        bc = aux_ps.tile([128, SQ], F32, tag="aux")
        nc.tensor.matmul(bc, _r(ones1), _r(rec))
        bc_s = misc.tile([128, SQ], F32, tag="bcs")
        nc.scalar.copy(bc_s, bc)
        nc.vector.tensor_mul(oT[h], pot, bc_s)


def _build_body(nc, tc, t, debug=False):
    from contextlib import ExitStack

    ctx = ExitStack()
    with ctx:
        consts = ctx.enter_context(tc.tile_pool(name="consts", bufs=1))
        persist = ctx.enter_context(tc.tile_pool(name="persist", bufs=1))
        misc = ctx.enter_context(tc.tile_pool(name="misc", bufs=2))
        # PSUM pools: aux first (lives through whole kernel), then phase pools.
        aux_ps = ctx.enter_context(tc.tile_pool(name="aux_ps", bufs=1, space="PSUM"))

        # ---- constants -----------------------------------------------------
        ones128f = consts.tile([128, 1], F32)
        nc.vector.memset(ones128f, 1.0)
        ones128 = consts.tile([128, 1], F32R)
        nc.scalar.copy(ones128, ones128f)
        ones1f = consts.tile([1, 128], F32)
        nc.vector.memset(ones1f, 1.0)
        ones1 = consts.tile([1, 128], F32R)
        nc.scalar.copy(ones1, ones1f)
        gq_s = consts.tile([1, QL], F32R)
        nc.sync.dma_start(out=gq_s, in_=t["gq"][:, :].bitcast(F32R))
        gkv_s = consts.tile([1, KV], F32R)
        nc.sync.dma_start(out=gkv_s, in_=t["gkv"][:, :].bitcast(F32R))
        eps_s = consts.tile([1, 1], F32)
        nc.vector.memset(eps_s, EPS)
        rotp_s = consts.tile([128, 128], BF16)
        nc.sync.dma_start(out=rotp_s, in_=t["rotp"][:, :])
        cosq_s = consts.tile([128, SQ], F32)
        nc.sync.dma_start(out=cosq_s, in_=t["cosq"][:, :])
        sinq_s = consts.tile([128, SQ], F32)
        nc.sync.dma_start(out=sinq_s, in_=t["sinq"][:, :])

        # ---- persistent tiles (bf16: matmul operands -> FWL weight loads) --
        kvcT = [persist.tile([128, S], BF16, tag=f"kvcT{c}", name=f"kvcT{c}") for c in range(NKV)]
        krT = persist.tile([128, S], BF16, tag="krT")
        qcT = [persist.tile([128, SQ], BF16, tag=f"qcT{c}", name=f"qcT{c}") for c in range(NQL)]
        oT = [persist.tile([128, SQ], BF16, tag=f"oT{h}", name=f"oT{h}") for h in range(H)]

        # ===================================================================
        # Phase 0: compress. kvcT/krT over full seq, qcT over own query rows.
        # ===================================================================
        with nc.named_scope("p0_compress", notify=True), \
             tc.tile_pool(name="misc0", bufs=2) as misc0, \
             tc.tile_pool(name="xtp", bufs=3) as xtp, \
             tc.tile_pool(name="wkvhold", bufs=1) as wkvhold, \
             tc.tile_pool(name="wstream", bufs=3) as wstream, \
             tc.tile_pool(name="acc_ps", bufs=6, space="PSUM") as acc_ps:
            # wckv/wkr are reused by all 4 seq blocks: load once, keep in SBUF
            wkv_h = [wkvhold.tile([128, KV], BF16, tag=f"wckv{d}", name=f"wckv{d}")
                     for d in range(16)]
            wkr_h = [wkvhold.tile([128, 128], BF16, tag=f"wkr{d}", name=f"wkr{d}")
                     for d in range(16)]
            for d in range(16):
                drow = slice(d * 128, (d + 1) * 128)
                nc.sync.dma_start(out=wkv_h[d], in_=t["wckv"][drow, :])
                nc.sync.dma_start(out=wkr_h[d], in_=t["wkr2"][drow, :])
            def p0_post(scol, kvraw, kraw):
                # rmsnorm over kv features (partition dim across the 4 chunks)
                ssq = aux_ps.tile([1, 512], F32, tag="aux")
                for c in range(NKV):
                    sq = misc0.tile([128, 512], F32R, tag="sq")
                    nc.scalar.square(sq, kvraw[c])
                    nc.tensor.matmul(ssq, _r(ones128), _r(sq),
                                     start=(c == 0), stop=(c == NKV - 1))
                rstd = misc0.tile([1, 512], F32R, tag="rstd")
                nc.scalar.activation(rstd, ssq,
                                     mybir.ActivationFunctionType.Sqrt,
                                     bias=eps_s[:, :], scale=1.0 / KV)
                with nc.allow_low_precision(reason="f32r is full fp32 bits"):
                    nc.vector.reciprocal(rstd, rstd)
                for c in range(NKV):
                    bc = aux_ps.tile([128, 512], F32, tag="aux")
                    nc.tensor.matmul(
                        bc, _r(gkv_s[:, c * 128:(c + 1) * 128]), _r(rstd))
                    bc_s = misc.tile([128, 512], F32, tag="bcs")
                    nc.scalar.copy(bc_s, bc)
                    nc.vector.tensor_mul(kvcT[c][:, scol], kvraw[c], bc_s)

                # rope on the (duplicated-rows) k_rope block
                ck = misc0.tile([128, 512], F32, tag="ck")
                nc.sync.dma_start(out=ck, in_=t["cosk"][:, scol])
                sk = misc0.tile([128, 512], F32, tag="sk")
                nc.sync.dma_start(out=sk, in_=t["sink"][:, scol])
                rot = aux_ps.tile([128, 512], F32, tag="aux")
                nc.tensor.matmul(rot, rotp_s, kraw)
                t1 = misc0.tile([128, 512], F32, tag="ropet1")
                nc.vector.tensor_mul(t1, kraw, ck)
                t2 = misc0.tile([128, 512], F32, tag="ropet2")
                nc.vector.tensor_mul(t2, rot, sk)
                nc.vector.tensor_add(krT[:, scol], t1, t2)

            p0_pending = None
            for sb in range(NS):
                scol = slice(sb * 512, (sb + 1) * 512)
                pkv = [acc_ps.tile([128, 512], F32, tag="acc", name="pkv") for _ in range(NKV)]
                pkr = acc_ps.tile([128, 512], F32, tag="acc")
                for d in range(16):
                    drow = slice(d * 128, (d + 1) * 128)
                    xt = xtp.tile([128, 512], BF16, tag="xt")
                    nc.sync.dma_start(out=xt, in_=t["xT"][drow, scol])
                    for c in range(NKV):
                        nc.tensor.matmul(
                            pkv[c], wkv_h[d][:, c * 128:(c + 1) * 128], xt,
                            start=(d == 0), stop=(d == 15))
                    nc.tensor.matmul(pkr, wkr_h[d], xt,
                                     start=(d == 0), stop=(d == 15))
                # drain psum to raw bf16 sbuf tiles (releases acc banks), then
                # run the PREVIOUS block's normalize behind this block's MMs.
                kvraw = [misc0.tile([128, 512], BF16, tag=f"kvraw{c}",
                                    name=f"kvraw{c}")
                         for c in range(NKV)]
                for c in range(NKV):
                    nc.scalar.copy(kvraw[c], pkv[c])
                kraw = misc0.tile([128, 512], BF16, tag="kraw")
                nc.scalar.copy(kraw, pkr)
                if p0_pending is not None:
                    p0_post(*p0_pending)
                p0_pending = (scol, kvraw, kraw)

            # qcT over own query rows
            pqc = [acc_ps.tile([128, 512], F32, tag="acc", name="pqc") for _ in range(NQL)]
            for d in range(16):
                drow = slice(d * 128, (d + 1) * 128)
                xt = xtp.tile([128, 512], BF16, tag="xt")
                nc.sync.dma_start(out=xt, in_=t["xqT"][drow, :])
                wq_t = wstream.tile([128, QL], BF16, tag="wcq")
                nc.sync.dma_start(out=wq_t, in_=t["wcq"][drow, :])
                for c in range(NQL):
                    nc.tensor.matmul(
                        pqc[c], wq_t[:, c * 128:(c + 1) * 128], xt,
                        start=(d == 0), stop=(d == 15))
            if p0_pending is not None:
                p0_post(*p0_pending)
                p0_pending = None
            ssq = aux_ps.tile([1, 512], F32, tag="aux")
            for c in range(NQL):
                sq = misc0.tile([128, 512], F32R, tag="sq")
                nc.scalar.square(sq, pqc[c])
                nc.tensor.matmul(ssq, _r(ones128), _r(sq),
                                 start=(c == 0), stop=(c == NQL - 1))
            rstd = misc0.tile([1, 512], F32R, tag="rstd")
            nc.scalar.activation(rstd, ssq, mybir.ActivationFunctionType.Sqrt,
                                 bias=eps_s[:, :], scale=1.0 / QL)
            with nc.allow_low_precision(reason="f32r is full fp32 bits"):
                nc.vector.reciprocal(rstd, rstd)
            for c in range(NQL):
                bc = aux_ps.tile([128, 512], F32, tag="aux")
                nc.tensor.matmul(bc, _r(gq_s[:, c * 128:(c + 1) * 128]), _r(rstd))
                bc_s = misc.tile([128, 512], F32, tag="bcs")
                nc.scalar.copy(bc_s, bc)
                nc.vector.tensor_mul(qcT[c], pqc[c], bc_s)

        # ===================================================================
        # Phase A: per head group -- decompress k/v/q, attention.
        # ===================================================================
        with nc.named_scope("pA_attn", notify=True), \
             tc.tile_pool(name="vpool", bufs=24) as vpool, \
             tc.tile_pool(name="khp", bufs=2) as khp, \
             tc.tile_pool(name="qnp", bufs=2) as qnp, \
             tc.tile_pool(name="qrp", bufs=2) as qrp, \
             tc.tile_pool(name="ptp", bufs=4) as ptp, \
             tc.tile_pool(name="denp", bufs=2) as denp, \
             tc.tile_pool(name="wdqp", bufs=6) as wdqp, \
             tc.tile_pool(name="wdqrp", bufs=6) as wdqrp, \
             tc.tile_pool(name="wdkp", bufs=4) as wdkp, \
             tc.tile_pool(name="wdvp", bufs=4) as wdvp, \
             tc.tile_pool(name="st_ps", bufs=3, space="PSUM") as st_ps, \
             tc.tile_pool(name="ot_ps", bufs=2, space="PSUM") as ot_ps, \
             tc.tile_pool(name="wk_ps", bufs=2, space="PSUM") as wk_ps:

            pending = []
            for g in range(GROUPS):
                gcol = slice(g * 512, (g + 1) * 512)
                # stream this group's decompress weights
                wdk_t = [wdkp.tile([128, 512], BF16, tag="wdk", name="wdk_t") for _ in range(NKV)]
                for c in range(NKV):
                    nc.sync.dma_start(
                        out=wdk_t[c], in_=t["wdk"][c * 128:(c + 1) * 128, gcol])
                wdv_t = [wdvp.tile([128, 512], BF16, tag="wdv", name="wdv_t") for _ in range(NKV)]
                for c in range(NKV):
                    nc.sync.dma_start(
                        out=wdv_t[c], in_=t["wdv"][c * 128:(c + 1) * 128, gcol])
                wdq_t = [wdqp.tile([128, 512], BF16, tag="wdq", name="wdq_t") for _ in range(NQL)]
                for c in range(NQL):
                    nc.sync.dma_start(
                        out=wdq_t[c], in_=t["wdq"][c * 128:(c + 1) * 128, gcol])
                grcol = slice(g * 256, (g + 1) * 256)
                wdqr_t = [wdqrp.tile([128, 256], BF16, tag="wdqr", name="wdqr_t") for _ in range(NQL)]
                for c in range(NQL):
                    nc.sync.dma_start(
                        out=wdqr_t[c], in_=t["wdqr"][c * 128:(c + 1) * 128, grcol])

                # v for all 4 heads of the group: moving = wdv (512 wide),
                # stationary = kvc seq-tile. Halves the matmul/LDW count vs
                # the per-pair 256-wide variant.
                vt = {}
                for st in range(NST):
                    pv = wk_ps.tile([128, 512], F32, tag="wk")
                    for c in range(NKV):
                        nc.tensor.matmul(
                            pv, kvcT[c][:, st * 128:(st + 1) * 128], wdv_t[c],
                            start=(c == 0), stop=(c == NKV - 1))
                    v_s = vpool.tile([128, 512], BF16, tag="v")
                    nc.scalar.copy(v_s, pv)
                    vt[st] = v_s

                for pair in range(GH // 2):
                    hA = g * GH + 2 * pair
                    hB = hA + 1
                    colA = slice((2 * pair) * 128, (2 * pair + 1) * 128)
                    colB = slice((2 * pair + 1) * 128, (2 * pair + 2) * 128)

                    # k_nope^T for both heads: [128 d, S]
                    khA = khp.tile([128, S], BF16, tag="kh")
                    khB = khp.tile([128, S], BF16, tag="kh")
                    for kh, hcol in ((khA, colA), (khB, colB)):
                        for blk in range(NS):
                            bcol = slice(blk * 512, (blk + 1) * 512)
                            pk = wk_ps.tile([128, 512], F32, tag="wk")
                            for c in range(NKV):
                                nc.tensor.matmul(
                                    pk, wdk_t[c][:, hcol], kvcT[c][:, bcol],
                                    start=(c == 0), stop=(c == NKV - 1))
                            nc.scalar.copy(kh[:, bcol], pk)

                    # q_nope^T for both heads: [128 d, SQ]
                    qnA = qnp.tile([128, SQ], BF16, tag="qn")
                    qnB = qnp.tile([128, SQ], BF16, tag="qn")
                    for qn, hcol in ((qnA, colA), (qnB, colB)):
                        pq = wk_ps.tile([128, SQ], F32, tag="wk")
                        for c in range(NQL):
                            nc.tensor.matmul(pq, wdq_t[c][:, hcol], qcT[c],
                                             start=(c == 0), stop=(c == NQL - 1))
                        nc.scalar.copy(qn, pq)

                    # q_rope for the pair (two heads stacked on partitions)
                    prcol = slice(pair * 128, (pair + 1) * 128)
                    pqr = wk_ps.tile([128, SQ], F32, tag="wk")
                    for c in range(NQL):
                        nc.tensor.matmul(
                            pqr, wdqr_t[c][:, prcol], qcT[c],
                            start=(c == 0), stop=(c == NQL - 1))
                    qraw = misc.tile([128, SQ], BF16, tag="qraw")
                    nc.scalar.copy(qraw, pqr)
                    rot = aux_ps.tile([128, SQ], F32, tag="aux")
                    nc.tensor.matmul(rot, rotp_s, qraw)
                    t1 = misc.tile([128, SQ], F32, tag="ropet1")
                    nc.vector.tensor_mul(t1, qraw, cosq_s)
                    t2 = misc.tile([128, SQ], F32, tag="ropet2")
                    nc.vector.tensor_mul(t2, rot, sinq_s)
                    qr_roped = qrp.tile([128, SQ], BF16, tag="qr")
                    nc.vector.tensor_add(qr_roped, t1, t2)

                    # normalize the PREVIOUS pair here: its den DVE chain has
                    # drained behind the decompress matmuls above, so the
                    # den1/bc matmuls no longer stall the PE queue.
                    _norm_flush(nc, misc, aux_ps, oT, ones128, ones1, pending)
                    pending.clear()

                    # attention for the pair. The two K=64 rope matmuls run on
                    # disjoint PE row-groups (base_partition 0 / 64) and
                    # overlap; stops are interleaved so exp can chase.
                    potA = ot_ps.tile([128, SQ], F32, tag="ot")
                    potB = ot_ps.tile([128, SQ], F32, tag="ot")
                    denA = denp.tile([128, SQ], F32R, tag="den")
                    denB = denp.tile([128, SQ], F32R, tag="den")
                    for kt in range(NST):
                        kcol = slice(kt * 128, (kt + 1) * 128)
                        pstA = st_ps.tile([128, SQ], F32, tag="st")
                        pstB = st_ps.tile([128, SQ], F32, tag="st")
                        nc.tensor.matmul(pstA, khA[:, kcol], qnA,
                                         start=True, stop=False)
                        nc.tensor.matmul(pstB, khB[:, kcol], qnB,
                                         start=True, stop=False)
                        nc.tensor.matmul(pstA, krT[0:64, kcol],
                                         qr_roped[0:64, :],
                                         start=False, stop=True)
                        nc.tensor.matmul(pstB, krT[64:128, kcol],
                                         qr_roped[64:128, :],
                                         start=False, stop=True)
                        ptA = ptp.tile([128, SQ], BF16, tag="pt")
                        nc.scalar.activation(ptA, pstA,
                                             mybir.ActivationFunctionType.Exp,
                                             scale=SCALE)
                        ptB = ptp.tile([128, SQ], BF16, tag="pt")
                        nc.scalar.activation(ptB, pstB,
                                             mybir.ActivationFunctionType.Exp,
                                             scale=SCALE)
                        if kt == 0:
                            nc.vector.tensor_copy(denA, ptA)
                            nc.vector.tensor_copy(denB, ptB)
                        else:
                            nc.vector.tensor_add(denA, denA, ptA)
                            nc.vector.tensor_add(denB, denB, ptB)
                        vs = vt[kt]
                        nc.tensor.matmul(
                            potA, vs[:, colA], ptA,
                            start=(kt == 0), stop=(kt == NST - 1))
                        nc.tensor.matmul(
                            potB, vs[:, colB], ptB,
                            start=(kt == 0), stop=(kt == NST - 1))

                    pending.append((hA, potA, denA))
                    pending.append((hB, potB, denB))

            _norm_flush(nc, misc, aux_ps, oT, ones128, ones1, pending)
            pending.clear()

        if debug:
            for c in range(NKV):
                nc.sync.dma_start(
                    out=t["dbg_kvcT"][c * 128:(c + 1) * 128, :], in_=kvcT[c])
            for c in range(NQL):
                nc.sync.dma_start(
                    out=t["dbg_qcT"][c * 128:(c + 1) * 128, :], in_=qcT[c])
            nc.sync.dma_start(out=t["dbg_krT"][:, :], in_=krT)
            for h in range(H):
                nc.sync.dma_start(
                    out=t["dbg_oT"][h * 128:(h + 1) * 128, :], in_=oT[h])

        # ===================================================================
        # Phase B: output projection, all 16 heads, PSUM-accumulated.
        # Loop order: wo tile loads once per (h, blk) and serves all 4 query
        # tiles (wo HBM traffic 16MB instead of 64MB).
        # ===================================================================
        NQT = SQ // 128
        with nc.named_scope("pB_outproj", notify=True), \
             tc.tile_pool(name="wop", bufs=4) as wop, \
             tc.tile_pool(name="outs", bufs=4) as outs, \
             tc.tile_pool(name="po_ps", bufs=4, space="PSUM") as po_ps:
            for blk in range(NS):
                bcol = slice(blk * 512, (blk + 1) * 512)
                po = [po_ps.tile([128, 512], F32, tag="po", name=f"po{qt}")
                      for qt in range(NQT)]
                for h in range(H):
                    wo_t = wop.tile([128, 512], BF16, tag="wo")
                    nc.sync.dma_start(
                        out=wo_t, in_=t["wo"][h * 128:(h + 1) * 128, bcol])
                    for qt in range(NQT):
                        nc.tensor.matmul(
                            po[qt], oT[h][:, qt * 128:(qt + 1) * 128], wo_t,
                            start=(h == 0), stop=(h == H - 1))
                for qt in range(NQT):
                    o_s = outs.tile([128, 512], F32, tag="os")
                    nc.scalar.copy(o_s, po[qt])
                    nc.sync.dma_start(
                        out=t["out"][qt * 128:(qt + 1) * 128, bcol], in_=o_s)


_NC_CACHE = None


def _get_nc():
    global _NC_CACHE
    if _NC_CACHE is None:
        _NC_CACHE = build_nc()
    return _NC_CACHE


def _rope_tables(positions):
    """cos/sin tables in transposed-packed layout [128, len(positions)]:
    rows 0:64 and 64:128 both hold the [RD, s] table (two rope vectors are
    stacked per 128 partitions)."""
    inv_freq = 1.0 / (10000.0 ** (np.arange(0, RD, 2, dtype=np.float32) / RD))
    ang = positions[:, None].astype(np.float32) * inv_freq[None, :]  # [s, 32]
    cos = np.concatenate([np.cos(ang), np.cos(ang)], axis=-1)        # [s, 64]
    sin = np.concatenate([np.sin(ang), np.sin(ang)], axis=-1)
    cosT = np.ascontiguousarray(cos.T)                               # [64, s]
    sinT = np.ascontiguousarray(sin.T)
    return (np.concatenate([cosT, cosT], axis=0),
            np.concatenate([sinT, sinT], axis=0))


def _rot_perm():
    m = np.zeros((128, 128), dtype=np.float32)
    for b0 in (0, 64):
        for i in range(32):
            m[b0 + i + 32, b0 + i] = -1.0   # rot[m] = -t[m+32], m < 32
            m[b0 + i, b0 + i + 32] = 1.0    # rot[m] = +t[m-32], m >= 32
    return m


def kernel(x, Wcq, g_q, Wdq, Wdqr, Wckv, g_kv, Wdk, Wdv, Wkr, Wo):
    import ml_dtypes

    bf16 = ml_dtypes.bfloat16
    nc = _get_nc()

    x = np.asarray(x, dtype=np.float32)
    xT = [np.ascontiguousarray(x[b].T).astype(bf16) for b in range(B)]  # [D, S]
    wkr2 = np.ascontiguousarray(
        np.concatenate([Wkr, Wkr], axis=1)).astype(bf16)  # [D, 128]
    cosk, sink = _rope_tables(np.arange(S))
    rotp = _rot_perm().astype(bf16)

    shared = {
        "wcq": np.ascontiguousarray(Wcq).astype(bf16),
        "wckv": np.ascontiguousarray(Wckv).astype(bf16),
        "wkr2": wkr2,
        "wdq": np.ascontiguousarray(Wdq).astype(bf16),
        "wdqr": np.ascontiguousarray(Wdqr).astype(bf16),
        "wdk": np.ascontiguousarray(Wdk).astype(bf16),
        "wdv": np.ascontiguousarray(Wdv).astype(bf16),
        "wo": np.ascontiguousarray(Wo).astype(bf16),
        "gq": np.ascontiguousarray(g_q, dtype=np.float32).reshape(1, QL),
        "gkv": np.ascontiguousarray(g_kv, dtype=np.float32).reshape(1, KV),
        "cosk": np.ascontiguousarray(cosk),
        "sink": np.ascontiguousarray(sink),
        "rotp": rotp,
    }

    in_maps = []
    for core in range(N_CORES):
        b, sl = core // 4, core % 4
        rows = np.arange(sl * SQ, (sl + 1) * SQ)
        cq, sq_t = _rope_tables(rows)
        m = dict(shared)
        m["xT"] = xT[b]
        m["xqT"] = np.ascontiguousarray(xT[b][:, sl * SQ:(sl + 1) * SQ])
        m["cosq"] = np.ascontiguousarray(cq)
        m["sinq"] = np.ascontiguousarray(sq_t)
        in_maps.append(m)

    trace = bool(int(os.environ.get("MLA_TRACE", "0")))
    res = run_bass_kernel_spmd(
        nc, in_maps, core_ids=list(range(N_CORES)), trace=trace,
        trace_cores=list(range(N_CORES)) if trace else None,
        stitch_traces=bool(int(os.environ.get("MLA_STITCH", "0"))),
        tmpdir=os.environ.get("MLA_TMPDIR") or None,
    )
    kernel.last_result = res

    out = np.empty((B, S, D), dtype=np.float32)
    for core in range(N_CORES):
        b, sl = core // 4, core % 4
        out[b, sl * SQ:(sl + 1) * SQ, :] = res.results[core]["out"]
    return out

